# revision 97
# baseline (speedup 1.0000x reference)
"""Multi-head causal attention (B=2, S=2048, D=1024, H=16) on 8 TRN2 NeuronCores.

Sharding: core c -> batch c//4, head-quarter c%4 (4 heads = 256 head dims).
Each core runs the full pipeline for its (batch, 4 heads):
  QKV projections -> causal softmax(QK^T/8) -> PV -> partial out-projection.
Host pre-transposes x / weight shards (so every DMA is contiguous) and
sums the 4 row-sharded out-projection partials per batch + bias.

Schedule notes (vs the 340us v1 baseline; this version measures ~197us):
  - scores for a head pair go to one [128, 2, 512] PSUM tile (2 banks) at
    PE row groups 0-63/64-127 (the pair's matmuls overlap in the array),
    and ONE batched exp covers both heads.
  - softmax denominators: 1/rowsum = exp(-ln(rowsum)) on ACT -- ln and exp
    share the natural_log_exp table set (no ACT table switching). The v1
    single-lane DVE reciprocal was 3.3us on the critical path and
    re-throttled the PE clock (HAM) 8x. The ENTIRE chain (ln, exp,
    broadcast matmul, scale multiplies) is deferred to the chunk boundary:
    ACT paces the attention phases, so any ACT op inside them stretches
    the kernel, while at the boundary ACT is idle and projection-chain PE
    work covers the chain's latency.
  - projection chains and out-projection units are pumped one per
    attention tile-pair so the PE always has exp-independent work; out-
    projection units are deferred up to two chunks so the longer later
    attention phases get enough filler (keeps the HAM clock gate warm).
  - input DMAs are one instruction per tensor/chunk (the sync sequencer
    costs ~600ns per DMA instruction) and dummy matmuls on the early tri
    load keep the PE active through the ~20us preamble load.
"""

import sys

import numpy as np

if "/opt/trn_rl_repo" not in sys.path:
    sys.path.insert(0, "/opt/trn_rl_repo")

import concourse.bass as bass
import concourse.mybir as mybir
import concourse.tile as tile
from concourse.bass import ts
from concourse.bass_utils import run_bass_kernel_spmd

P = 128          # partitions
S = 2048         # sequence length
DD = 1024        # model dim
DC = DD // P     # d-model chunks (8)
E = 256          # head dims per core (4 heads x 64)
H4 = 4           # heads per core
HD = 64
NQ = 4           # q chunks of 512
QC = 512
KT = S // P      # k tiles (16)
FD = 512         # matmul free dim

F32 = mybir.dt.float32
FR = mybir.dt.float32r
EXP = mybir.ActivationFunctionType.Exp
LOG = mybir.ActivationFunctionType.Ln
MUL = mybir.AluOpType.mult


def _emit(tc, nc, xT_d, wq_d, wk_d, wv_d, wo_d, tri_d, out_d):
    with (
        tc.tile_pool(name="const", bufs=1) as const,
        tc.tile_pool(name="attn", bufs=3) as attn_pool,
        tc.tile_pool(name="qt", bufs=2) as qt_pool,
        tc.tile_pool(name="ct", bufs=3) as ct_pool,
        tc.tile_pool(name="ctxu", bufs=4) as ctxu_pool,
        tc.tile_pool(name="rsp", bufs=4) as rs_pool,
        tc.tile_pool(name="ostage", bufs=3) as ostage,
        tc.tile_pool(name="pmm", bufs=2, space="PSUM") as pmm,
        tc.tile_pool(name="pacc", bufs=2, space="PSUM") as pacc,
        tc.tile_pool(name="psc", bufs=2, space="PSUM") as psc,
    ):
        # x is chunk-major [P, NQ, DC, QC]: each n-chunk is one contiguous
        # 16KB-per-partition block, so its load is a single fat-descriptor
        # DMA instead of being paced by 2KB descriptors.
        xT = const.tile([P, NQ, DC, QC], FR)
        wq = const.tile([P, DC, E], FR)
        wk = const.tile([P, DC, E], FR)
        wv = const.tile([P, DC, E], FR)
        wo = const.tile([P, 2, DD], FR)
        tri = const.tile([P, P], FR)
        kT = const.tile([P, 2, S], FR)
        vS = const.tile([P, KT, H4, HD + 1], FR)
        # q / ctx live only for one n-chunk each -> rotate through pools
        qTs = {}
        cTs = {}

        # One DMA instruction per tensor/chunk: the sync sequencer costs
        # ~600ns per DMA instruction, so per-block loads would pace the
        # whole preamble at ~1.3us/block instead of HBM bandwidth.
        with tc.high_priority():
            nc.sync.dma_start(tri[:], tri_d[:])
            nc.sync.dma_start(wq[:], wq_d[:])
            nc.sync.dma_start(xT[:, 0], xT_d[:, 0])
            nc.sync.dma_start(wk[:], wk_d[:])
            nc.sync.dma_start(wv[:], wv_d[:])
            for j in range(1, NQ):
                nc.sync.dma_start(xT[:, j], xT_d[:, j])
            nc.sync.dma_start(wo[:], wo_d[:])

        # f32 constants (Memset cannot encode float32r; copies can round to it)
        ones4 = const.tile([P, H4], F32)
        nc.vector.memset(ones4[:], 1.0)
        zcol = const.tile([P, 384], F32)
        nc.vector.memset(zcol[:], 0.0)
        onesf = const.tile([1, HD], F32)
        nc.vector.memset(onesf[:], 1.0)
        ones64 = const.tile([1, HD], FR)
        nc.vector.tensor_copy(ones64[:], onesf[:])
        # ones column of V_ext (row sums of exp-scores come out of the PV matmul)
        for i in range(KT):
            nc.vector.tensor_copy(vS[:, i, :, HD], ones4[:])

        ncopy = 0

        def psum_copy(dst, src):
            # alternate PSUM->SBUF copies between ScalarE and VectorE
            nonlocal ncopy
            eng = nc.scalar if ncopy % 2 == 0 else nc.vector
            if eng is nc.scalar:
                eng.copy(dst, src)
            else:
                eng.tensor_copy(dst, src)
            ncopy += 1

        def qk_unit(j, w_s, et):
            def unit():
                ps = pmm.tile([P, FD], F32, tag="mm", name="ps_proj")
                for c in range(DC):
                    nc.tensor.matmul(
                        ps[:],
                        lhsT=w_s[:, c, ts(et, P)],
                        rhs=xT[:, j, c, :],
                        start=(c == 0),
                        stop=(c == DC - 1),
                    )
                if w_s is wq:
                    nc.vector.tensor_copy(qTs[j][:, et, :], ps[:])
                else:
                    nc.vector.tensor_copy(kT[:, et, ts(j, QC)], ps[:])

            return unit

        def v_unit(j, nt):
            def unit():
                psv = pmm.tile([P, FD], F32, tag="mm", name="ps_v")
                for c in range(DC):
                    nc.tensor.matmul(
                        psv[:, :E],
                        lhsT=xT[:, j, c, ts(nt - 4 * j, P)],
                        rhs=wv[:, c, :],
                        start=(c == 0),
                        stop=(c == DC - 1),
                    )
                nc.vector.tensor_copy(
                    vS[:, nt, :, 0:HD],
                    psv[:, :E].rearrange("p (h d) -> p h d", h=H4),
                )

            return unit

        def proj_units(j):
            # chunk-j projections; qT tile allocated eagerly so units can be
            # pumped out of order relative to attention emission
            qTs[j] = qt_pool.tile([P, 2, QC], FR, tag="qt", name="qT")
            units = [qk_unit(j, w, et) for w in (wq, wk) for et in range(2)]
            units += [v_unit(j, nt) for nt in range(4 * j, 4 * j + 4)]
            return units

        # deferred PE work (projection / out-projection units), pumped one
        # unit per attention tile pair so the PE queue always has
        # exp-independent work. RESERVE units stay queued to cover each
        # chunk's final normalization chain (ACT latency) with PE work.
        pump_q = []
        tail_flush = [False]
        RESERVE = 6

        def pump(n, reserve=RESERVE):
            for _ in range(n):
                if len(pump_q) > reserve:
                    pump_q.pop(0)()

        def flush(n):
            for _ in range(n):
                if pump_q:
                    pump_q.pop(0)()

        def drain_pair(j, hp, pvs):
            """Copy ctx AND the rowsum row out of PSUM (frees the PV banks
            immediately). The whole 1/rowsum = exp(-ln(rowsum)) chain (both
            functions live in the natural_log_exp table set -> no table
            switching) is DEFERRED with the broadcast matmul + scale
            multiplies: ACT paces the attention phases, so running the
            ln/exp there would stretch them -- at the chunk boundary ACT is
            idle and proj-chain PE work covers the chain's latency."""
            ctxu = []
            for hh in range(2):
                cu = ctxu_pool.tile([HD + 1, QC], F32, tag="cu", name="cu")
                nc.vector.tensor_copy(cu[:], pvs[hh][0:HD + 1, :])
                ctxu.append(cu)

            def pe_part():
                for hh in range(2):
                    lt = rs_pool.tile([1, QC], F32, tag="lt", name="lntmp")
                    nc.scalar.activation(lt[:], ctxu[hh][HD:HD + 1, :], LOG)
                    rf = rs_pool.tile([1, QC], FR, tag="rsf", name="rsf")
                    nc.scalar.activation(rf[:], lt[:], EXP, scale=-1.0)
                    # broadcast across the head's 64 partitions via a
                    # rank-1 ones-matmul
                    bc_ps = pmm.tile([HD, QC], F32, tag="mm", name="bc_ps")
                    nc.tensor.matmul(
                        bc_ps[:], lhsT=ones64[:], rhs=rf[:],
                        start=True, stop=True,
                    )
                    nc.vector.tensor_tensor(
                        cTs[j][HD * hh:HD * hh + HD, hp, :],
                        ctxu[hh][0:HD, :],
                        bc_ps[:],
                        MUL,
                    )

            return pe_part

        def attention(j):
            nk = 4 * (j + 1)
            rsv = 12 if j == 3 else RESERVE  # extra tail cover: the final
            # chunk's deferred ACT chains + out-projection epilogue have no
            # following attention phase to hide in
            cTs[j] = ct_pool.tile([P, 2, QC], FR, tag="ct", name="cT")
            drains = []
            for hp in range(2):
                h0, h1 = 2 * hp, 2 * hp + 1
                pvs = [
                    pacc.tile([HD + 1, QC], F32, tag="pv", name=f"pv{h}")
                    for h in (h0, h1)
                ]
                for i in range(nk):
                    sc2 = psc.tile([P, 2, QC], F32, tag="sc", name="sc2")
                    # head pair scores: rows 0-63 and 64-127 of the PE array
                    nc.tensor.matmul(
                        sc2[:, 0, :],
                        lhsT=kT[0:HD, hp, ts(i, P)],
                        rhs=qTs[j][0:HD, hp, :],
                        start=True,
                        stop=True,
                    )
                    nc.tensor.matmul(
                        sc2[:, 1, :],
                        lhsT=kT[HD:P, hp, ts(i, P)],
                        rhs=qTs[j][HD:P, hp, :],
                        start=True,
                        stop=True,
                    )
                    at2 = attn_pool.tile([P, 2, QC], FR, tag="at", name="at2")
                    coff = P * (i - 4 * j)
                    if coff < 0:
                        nc.scalar.activation(at2[:], sc2[:], EXP)
                    else:
                        if coff > 0:
                            for hh in range(2):
                                nc.vector.tensor_copy(
                                    at2[:, hh, 0:coff], zcol[:, 0:coff]
                                )
                        nc.scalar.activation(
                            at2[:, :, coff:QC], sc2[:, :, coff:QC], EXP
                        )
                        for hh in range(2):
                            nc.vector.tensor_tensor(
                                at2[:, hh, coff:coff + P],
                                at2[:, hh, coff:coff + P],
                                tri[:],
                                MUL,
                            )
                    for hh in range(2):
                        nc.tensor.matmul(
                            pvs[hh][:],
                            lhsT=vS[:, i, (h0, h1)[hh], :],
                            rhs=at2[:, hh, :],
                            start=(i == 0),
                            stop=(i == nk - 1),
                        )
                    pump(1, rsv)
                drains.append(drain_pair(j, hp, pvs))
            return drains

        def outproj_units(j):
            units = []
            for nt in range(4 * j, 4 * j + 4):
                for fc in range(2):

                    def unit(nt=nt, fc=fc):
                        po = pmm.tile([P, FD], F32, tag="mm", name="ps_out")
                        for c in range(2):
                            nc.tensor.matmul(
                                po[:],
                                lhsT=cTs[j][:, c, ts(nt - 4 * j, P)],
                                rhs=wo[:, c, ts(fc, FD)],
                                start=(c == 0),
                                stop=(c == 1),
                            )
                        ob = ostage.tile([P, FD], F32, tag="ob", name="ob")
                        if tail_flush[0]:
                            psum_copy(ob[:], po[:])  # ACT is idle at the tail
                        else:
                            nc.vector.tensor_copy(ob[:], po[:])
                        nc.sync.dma_start(out_d[ts(nt, P), ts(fc, FD)], ob[:])

                    units.append(unit)
            return units

        # Warm-up: the PE sits idle for the first ~20us waiting on the x/w
        # DMAs; dummy matmuls on the (early, tiny) tri load spend that
        # otherwise-idle time keeping the PE active.
        for w in range(40):
            wp = pmm.tile([P, P], F32, tag="mm", name="warm")
            nc.tensor.matmul(
                wp[:], lhsT=tri[:], rhs=tri[:], start=True, stop=True
            )

        # Filler distribution: each attention phase pumps one deferred unit
        # per tile pair so the PE never idles long enough for the HAM clock
        # gate to re-throttle. Later chunks have more pairs, so out-projection
        # units are deferred up to two chunks to even out the filler supply.
        for u in proj_units(0):
            u()
        deferred = []
        for j in range(NQ):
            if j + 1 < NQ:
                # chunk j+1 projections pump into attention(j)'s stalls
                pump_q.extend(proj_units(j + 1))
            if j == 3:
                pump_q.extend(deferred)
                deferred = []
            dps = attention(j)
            flush(len(pump_q))  # whatever attention didn't absorb
            for dp in dps:
                dp()
            ou = outproj_units(j)
            if j in (0, 1):
                deferred.extend(ou)  # runs during attention(2) / attention(3)
            else:
                pump_q.extend(ou)
            if j == 1:
                pump_q.extend(deferred[:8])  # outproj(0) -> attention(2)
                deferred = deferred[8:]
        tail_flush[0] = True
        flush(len(pump_q))


def _split_multi_waits(nc):
    """The TRN2 instruction encoding carries ONE sync-wait slot; this walrus
    build rejects instructions with more. Hoist extra waits onto standalone
    EventSemaphore instructions immediately before (same engine queue, same
    semantics)."""
    n = 0
    for f in nc.m.functions:
        for b in f.blocks:
            out = []
            for i in list(b.instructions):
                si = i.sync_info
                if si is not None and len(si.on_wait) > 1:
                    waits = list(si.on_wait)
                    for w in waits[:-1]:
                        n += 1
                        out.append(
                            mybir.InstEventSemaphore(
                                name=f"I-wsplit{n}",
                                engine=i.engine,
                                ins=[],
                                outs=[],
                                sync_info=mybir.SyncInfo(on_wait=[w], on_update=[]),
                            )
                        )
                    i.sync_info = mybir.SyncInfo(
                        on_wait=[waits[-1]], on_update=list(si.on_update)
                    )
                out.append(i)
            b.instructions = out


def build_nc(split_waits=True):
    nc = bass.Bass("TRN2", target_bir_lowering=False, debug=False)
    xT_d = nc.dram_tensor("xT", [P, NQ, DC, QC], FR, kind="ExternalInput").ap()
    wq_d = nc.dram_tensor("wqT", [P, DC, E], FR, kind="ExternalInput").ap()
    wk_d = nc.dram_tensor("wkT", [P, DC, E], FR, kind="ExternalInput").ap()
    wv_d = nc.dram_tensor("wvT", [P, DC, E], FR, kind="ExternalInput").ap()
    wo_d = nc.dram_tensor("woT", [P, 2, DD], FR, kind="ExternalInput").ap()
    tri_d = nc.dram_tensor("tri", [P, P], FR, kind="ExternalInput").ap()
    out_d = nc.dram_tensor("out", [S, DD], F32, kind="ExternalOutput").ap()
    with tile.TileContext(nc) as tc:
        _emit(tc, nc, xT_d, wq_d, wk_d, wv_d, wo_d, tri_d, out_d)
    if split_waits:
        _split_multi_waits(nc)
    return nc


def _strip(a, chunks):
    """[D, N] -> [128, D//128, N] with partition-major layout, contiguous."""
    d, n = a.shape
    return np.ascontiguousarray(
        a.reshape(chunks, P, n).transpose(1, 0, 2), dtype=np.float32
    )


def make_in_maps(x, Wq, Wk, Wv, Wo):
    tri = np.ascontiguousarray(np.triu(np.ones((P, P), np.float32)))
    in_maps = []
    for c in range(8):
        b, g = c // 4, c % 4
        sl = slice(E * g, E * (g + 1))
        xs = _strip(x[b].T.astype(np.float32), DC)  # [P, DC, S]
        xs = np.ascontiguousarray(
            xs.reshape(P, DC, NQ, QC).transpose(0, 2, 1, 3)
        )  # [P, NQ, DC, QC], chunk-major
        in_maps.append(
            {
                "xT": xs,
                "wqT": _strip((Wq[sl, :] * 0.125).T.astype(np.float32), DC),
                "wkT": _strip(Wk[sl, :].T.astype(np.float32), DC),
                "wvT": _strip(Wv[sl, :].T.astype(np.float32), DC),
                "woT": _strip(Wo[:, sl].T.astype(np.float32), 2),
                "tri": tri,
            }
        )
    return in_maps


def kernel(x, Wq, Wk, Wv, Wo, bo, _run_kwargs=None):
    x, Wq, Wk, Wv, Wo, bo = (
        np.asarray(a, dtype=np.float32) for a in (x, Wq, Wk, Wv, Wo, bo)
    )
    nc = build_nc()
    in_maps = make_in_maps(x, Wq, Wk, Wv, Wo)
    res = run_bass_kernel_spmd(
        nc, in_maps, core_ids=list(range(8)), **(_run_kwargs or {})
    )
    out = np.zeros((2, S, DD), dtype=np.float32)
    for c in range(8):
        out[c // 4] += res.results[c]["out"]
    out += bo[None, None, :]
    if _run_kwargs:
        kernel.last_results = res
    return out


# revision 98
# speedup vs baseline: 1.0148x; 1.0148x over previous
"""Multi-head causal attention (B=2, S=2048, D=1024, H=16) on 8 TRN2 NeuronCores.

Sharding: core c -> batch c//4, head-quarter c%4 (4 heads = 256 head dims).
Each core runs the full pipeline for its (batch, 4 heads):
  QKV projections -> causal softmax(QK^T/8) -> PV -> partial out-projection.
Host pre-transposes x / weight shards (so every DMA is contiguous) and
sums the 4 row-sharded out-projection partials per batch + bias.

Schedule notes (vs the 340us v1 baseline; this version measures ~197us):
  - scores for a head pair go to one [128, 2, 512] PSUM tile (2 banks) at
    PE row groups 0-63/64-127 (the pair's matmuls overlap in the array),
    and ONE batched exp covers both heads.
  - softmax denominators: 1/rowsum = exp(-ln(rowsum)) on ACT -- ln and exp
    share the natural_log_exp table set (no ACT table switching). The v1
    single-lane DVE reciprocal was 3.3us on the critical path and
    re-throttled the PE clock (HAM) 8x. The ENTIRE chain (ln, exp,
    broadcast matmul, scale multiplies) is deferred to the chunk boundary:
    ACT paces the attention phases, so any ACT op inside them stretches
    the kernel, while at the boundary ACT is idle and projection-chain PE
    work covers the chain's latency.
  - projection chains and out-projection units are pumped one per
    attention tile-pair so the PE always has exp-independent work; out-
    projection units are deferred up to two chunks so the longer later
    attention phases get enough filler (keeps the HAM clock gate warm).
  - input DMAs are one instruction per tensor/chunk (the sync sequencer
    costs ~600ns per DMA instruction) and dummy matmuls on the early tri
    load keep the PE active through the ~20us preamble load.
"""

import sys

import numpy as np

if "/opt/trn_rl_repo" not in sys.path:
    sys.path.insert(0, "/opt/trn_rl_repo")

import concourse.bass as bass
import concourse.mybir as mybir
import concourse.tile as tile
from concourse.bass import ts
from concourse.bass_utils import run_bass_kernel_spmd

P = 128          # partitions
S = 2048         # sequence length
DD = 1024        # model dim
DC = DD // P     # d-model chunks (8)
E = 256          # head dims per core (4 heads x 64)
H4 = 4           # heads per core
HD = 64
NQ = 4           # q chunks of 512
QC = 512
KT = S // P      # k tiles (16)
FD = 512         # matmul free dim

F32 = mybir.dt.float32
FR = mybir.dt.float32r
EXP = mybir.ActivationFunctionType.Exp
LOG = mybir.ActivationFunctionType.Ln
MUL = mybir.AluOpType.mult


def _emit(tc, nc, xT_d, wq_d, wk_d, wv_d, wo_d, tri_d, out_d):
    with (
        tc.tile_pool(name="const", bufs=1) as const,
        tc.tile_pool(name="attn", bufs=4) as attn_pool,
        tc.tile_pool(name="qt", bufs=2) as qt_pool,
        tc.tile_pool(name="ct", bufs=3) as ct_pool,
        tc.tile_pool(name="ctxu", bufs=4) as ctxu_pool,
        tc.tile_pool(name="rsp", bufs=4) as rs_pool,
        tc.tile_pool(name="ostage", bufs=3) as ostage,
        tc.tile_pool(name="pmm", bufs=2, space="PSUM") as pmm,
        tc.tile_pool(name="pacc", bufs=2, space="PSUM") as pacc,
        tc.tile_pool(name="psc", bufs=2, space="PSUM") as psc,
    ):
        # x is chunk-major [P, NQ, DC, QC]: each n-chunk is one contiguous
        # 16KB-per-partition block, so its load is a single fat-descriptor
        # DMA instead of being paced by 2KB descriptors.
        xT = const.tile([P, NQ, DC, QC], FR)
        wq = const.tile([P, DC, E], FR)
        wk = const.tile([P, DC, E], FR)
        wv = const.tile([P, DC, E], FR)
        wo = const.tile([P, 2, DD], FR)
        tri = const.tile([P, P], FR)
        kT = const.tile([P, 2, S], FR)
        vS = const.tile([P, KT, H4, HD + 1], FR)
        # q / ctx live only for one n-chunk each -> rotate through pools
        qTs = {}
        cTs = {}

        # One DMA instruction per tensor/chunk: the sync sequencer costs
        # ~600ns per DMA instruction, so per-block loads would pace the
        # whole preamble at ~1.3us/block instead of HBM bandwidth.
        with tc.high_priority():
            nc.sync.dma_start(tri[:], tri_d[:])
            nc.sync.dma_start(wq[:], wq_d[:])
            nc.sync.dma_start(xT[:, 0], xT_d[:, 0])
            nc.sync.dma_start(wk[:], wk_d[:])
            nc.sync.dma_start(wv[:], wv_d[:])
            for j in range(1, NQ):
                nc.sync.dma_start(xT[:, j], xT_d[:, j])
            nc.sync.dma_start(wo[:], wo_d[:])

        # f32 constants (Memset cannot encode float32r; copies can round to it)
        ones4 = const.tile([P, H4], F32)
        nc.vector.memset(ones4[:], 1.0)
        zcol = const.tile([P, 384], F32)
        nc.vector.memset(zcol[:], 0.0)
        onesf = const.tile([1, HD], F32)
        nc.vector.memset(onesf[:], 1.0)
        ones64 = const.tile([1, HD], FR)
        nc.vector.tensor_copy(ones64[:], onesf[:])
        # ones column of V_ext (row sums of exp-scores come out of the PV matmul)
        for i in range(KT):
            nc.vector.tensor_copy(vS[:, i, :, HD], ones4[:])

        ncopy = 0

        def psum_copy(dst, src):
            # alternate PSUM->SBUF copies between ScalarE and VectorE
            nonlocal ncopy
            eng = nc.scalar if ncopy % 2 == 0 else nc.vector
            if eng is nc.scalar:
                eng.copy(dst, src)
            else:
                eng.tensor_copy(dst, src)
            ncopy += 1

        def qk_unit(j, w_s, et):
            def unit():
                ps = pmm.tile([P, FD], F32, tag="mm", name="ps_proj")
                for c in range(DC):
                    nc.tensor.matmul(
                        ps[:],
                        lhsT=w_s[:, c, ts(et, P)],
                        rhs=xT[:, j, c, :],
                        start=(c == 0),
                        stop=(c == DC - 1),
                    )
                if w_s is wq:
                    nc.vector.tensor_copy(qTs[j][:, et, :], ps[:])
                else:
                    nc.vector.tensor_copy(kT[:, et, ts(j, QC)], ps[:])

            return unit

        def v_unit(j, nt):
            def unit():
                psv = pmm.tile([P, FD], F32, tag="mm", name="ps_v")
                for c in range(DC):
                    nc.tensor.matmul(
                        psv[:, :E],
                        lhsT=xT[:, j, c, ts(nt - 4 * j, P)],
                        rhs=wv[:, c, :],
                        start=(c == 0),
                        stop=(c == DC - 1),
                    )
                nc.vector.tensor_copy(
                    vS[:, nt, :, 0:HD],
                    psv[:, :E].rearrange("p (h d) -> p h d", h=H4),
                )

            return unit

        def proj_units(j):
            # chunk-j projections; qT tile allocated eagerly so units can be
            # pumped out of order relative to attention emission
            qTs[j] = qt_pool.tile([P, 2, QC], FR, tag="qt", name="qT")
            units = [qk_unit(j, w, et) for w in (wq, wk) for et in range(2)]
            units += [v_unit(j, nt) for nt in range(4 * j, 4 * j + 4)]
            return units

        # deferred PE work (projection / out-projection units), pumped one
        # unit per attention tile pair so the PE queue always has
        # exp-independent work. RESERVE units stay queued to cover each
        # chunk's final normalization chain (ACT latency) with PE work.
        pump_q = []
        tail_flush = [False]
        RESERVE = 6

        def pump(n, reserve=RESERVE):
            for _ in range(n):
                if len(pump_q) > reserve:
                    pump_q.pop(0)()

        def flush(n):
            for _ in range(n):
                if pump_q:
                    pump_q.pop(0)()

        def drain_pair(j, hp, pvs):
            """Copy ctx AND the rowsum row out of PSUM (frees the PV banks
            immediately). The whole 1/rowsum = exp(-ln(rowsum)) chain (both
            functions live in the natural_log_exp table set -> no table
            switching) is DEFERRED with the broadcast matmul + scale
            multiplies: ACT paces the attention phases, so running the
            ln/exp there would stretch them -- at the chunk boundary ACT is
            idle and proj-chain PE work covers the chain's latency."""
            ctxu = []
            for hh in range(2):
                cu = ctxu_pool.tile([HD + 1, QC], F32, tag="cu", name="cu")
                nc.vector.tensor_copy(cu[:], pvs[hh][0:HD + 1, :])
                ctxu.append(cu)

            def pe_part():
                for hh in range(2):
                    lt = rs_pool.tile([1, QC], F32, tag="lt", name="lntmp")
                    nc.scalar.activation(lt[:], ctxu[hh][HD:HD + 1, :], LOG)
                    rf = rs_pool.tile([1, QC], FR, tag="rsf", name="rsf")
                    nc.scalar.activation(rf[:], lt[:], EXP, scale=-1.0)
                    # broadcast across the head's 64 partitions via a
                    # rank-1 ones-matmul
                    bc_ps = pmm.tile([HD, QC], F32, tag="mm", name="bc_ps")
                    nc.tensor.matmul(
                        bc_ps[:], lhsT=ones64[:], rhs=rf[:],
                        start=True, stop=True,
                    )
                    nc.vector.tensor_tensor(
                        cTs[j][HD * hh:HD * hh + HD, hp, :],
                        ctxu[hh][0:HD, :],
                        bc_ps[:],
                        MUL,
                    )

            return pe_part

        def attention(j):
            nk = 4 * (j + 1)
            rsv = 12 if j == 3 else RESERVE  # extra tail cover: the final
            # chunk's deferred ACT chains + out-projection epilogue have no
            # following attention phase to hide in
            cTs[j] = ct_pool.tile([P, 2, QC], FR, tag="ct", name="cT")
            drains = []
            for hp in range(2):
                h0, h1 = 2 * hp, 2 * hp + 1
                pvs = [
                    pacc.tile([HD + 1, QC], F32, tag="pv", name=f"pv{h}")
                    for h in (h0, h1)
                ]
                for i in range(nk):
                    sc2 = psc.tile([P, 2, QC], F32, tag="sc", name="sc2")
                    # head pair scores: rows 0-63 and 64-127 of the PE array
                    nc.tensor.matmul(
                        sc2[:, 0, :],
                        lhsT=kT[0:HD, hp, ts(i, P)],
                        rhs=qTs[j][0:HD, hp, :],
                        start=True,
                        stop=True,
                    )
                    nc.tensor.matmul(
                        sc2[:, 1, :],
                        lhsT=kT[HD:P, hp, ts(i, P)],
                        rhs=qTs[j][HD:P, hp, :],
                        start=True,
                        stop=True,
                    )
                    at2 = attn_pool.tile([P, 2, QC], FR, tag="at", name="at2")
                    coff = P * (i - 4 * j)
                    if coff < 0:
                        nc.scalar.activation(at2[:], sc2[:], EXP)
                    else:
                        if coff > 0:
                            for hh in range(2):
                                nc.vector.tensor_copy(
                                    at2[:, hh, 0:coff], zcol[:, 0:coff]
                                )
                        nc.scalar.activation(
                            at2[:, :, coff:QC], sc2[:, :, coff:QC], EXP
                        )
                        for hh in range(2):
                            nc.vector.tensor_tensor(
                                at2[:, hh, coff:coff + P],
                                at2[:, hh, coff:coff + P],
                                tri[:],
                                MUL,
                            )
                    for hh in range(2):
                        nc.tensor.matmul(
                            pvs[hh][:],
                            lhsT=vS[:, i, (h0, h1)[hh], :],
                            rhs=at2[:, hh, :],
                            start=(i == 0),
                            stop=(i == nk - 1),
                        )
                    pump(1, rsv)
                drains.append(drain_pair(j, hp, pvs))
            return drains

        def outproj_units(j):
            units = []
            for nt in range(4 * j, 4 * j + 4):
                for fc in range(2):

                    def unit(nt=nt, fc=fc):
                        po = pmm.tile([P, FD], F32, tag="mm", name="ps_out")
                        for c in range(2):
                            nc.tensor.matmul(
                                po[:],
                                lhsT=cTs[j][:, c, ts(nt - 4 * j, P)],
                                rhs=wo[:, c, ts(fc, FD)],
                                start=(c == 0),
                                stop=(c == 1),
                            )
                        ob = ostage.tile([P, FD], F32, tag="ob", name="ob")
                        if tail_flush[0]:
                            psum_copy(ob[:], po[:])  # ACT is idle at the tail
                        else:
                            nc.vector.tensor_copy(ob[:], po[:])
                        nc.sync.dma_start(out_d[ts(nt, P), ts(fc, FD)], ob[:])

                    units.append(unit)
            return units

        # Warm-up: the PE sits idle for the first ~20us waiting on the x/w
        # DMAs; dummy matmuls on the (early, tiny) tri load spend that
        # otherwise-idle time keeping the PE active.
        for w in range(40):
            wp = pmm.tile([P, P], F32, tag="mm", name="warm")
            nc.tensor.matmul(
                wp[:], lhsT=tri[:], rhs=tri[:], start=True, stop=True
            )

        # Filler distribution: each attention phase pumps one deferred unit
        # per tile pair so the PE never idles long enough for the HAM clock
        # gate to re-throttle. Later chunks have more pairs, so out-projection
        # units are deferred up to two chunks to even out the filler supply.
        for u in proj_units(0):
            u()
        deferred = []
        for j in range(NQ):
            if j + 1 < NQ:
                # chunk j+1 projections pump into attention(j)'s stalls
                pump_q.extend(proj_units(j + 1))
            if j == 3:
                pump_q.extend(deferred)
                deferred = []
            dps = attention(j)
            flush(len(pump_q))  # whatever attention didn't absorb
            for dp in dps:
                dp()
            ou = outproj_units(j)
            if j in (0, 1):
                deferred.extend(ou)  # runs during attention(2) / attention(3)
            else:
                pump_q.extend(ou)
            if j == 1:
                pump_q.extend(deferred[:8])  # outproj(0) -> attention(2)
                deferred = deferred[8:]
        tail_flush[0] = True
        flush(len(pump_q))


def _split_multi_waits(nc):
    """The TRN2 instruction encoding carries ONE sync-wait slot; this walrus
    build rejects instructions with more. Hoist extra waits onto standalone
    EventSemaphore instructions immediately before (same engine queue, same
    semantics)."""
    n = 0
    for f in nc.m.functions:
        for b in f.blocks:
            out = []
            for i in list(b.instructions):
                si = i.sync_info
                if si is not None and len(si.on_wait) > 1:
                    waits = list(si.on_wait)
                    for w in waits[:-1]:
                        n += 1
                        out.append(
                            mybir.InstEventSemaphore(
                                name=f"I-wsplit{n}",
                                engine=i.engine,
                                ins=[],
                                outs=[],
                                sync_info=mybir.SyncInfo(on_wait=[w], on_update=[]),
                            )
                        )
                    i.sync_info = mybir.SyncInfo(
                        on_wait=[waits[-1]], on_update=list(si.on_update)
                    )
                out.append(i)
            b.instructions = out


def build_nc(split_waits=True):
    nc = bass.Bass("TRN2", target_bir_lowering=False, debug=False)
    xT_d = nc.dram_tensor("xT", [P, NQ, DC, QC], FR, kind="ExternalInput").ap()
    wq_d = nc.dram_tensor("wqT", [P, DC, E], FR, kind="ExternalInput").ap()
    wk_d = nc.dram_tensor("wkT", [P, DC, E], FR, kind="ExternalInput").ap()
    wv_d = nc.dram_tensor("wvT", [P, DC, E], FR, kind="ExternalInput").ap()
    wo_d = nc.dram_tensor("woT", [P, 2, DD], FR, kind="ExternalInput").ap()
    tri_d = nc.dram_tensor("tri", [P, P], FR, kind="ExternalInput").ap()
    out_d = nc.dram_tensor("out", [S, DD], F32, kind="ExternalOutput").ap()
    with tile.TileContext(nc) as tc:
        _emit(tc, nc, xT_d, wq_d, wk_d, wv_d, wo_d, tri_d, out_d)
    if split_waits:
        _split_multi_waits(nc)
    return nc


def _strip(a, chunks):
    """[D, N] -> [128, D//128, N] with partition-major layout, contiguous."""
    d, n = a.shape
    return np.ascontiguousarray(
        a.reshape(chunks, P, n).transpose(1, 0, 2), dtype=np.float32
    )


def make_in_maps(x, Wq, Wk, Wv, Wo):
    tri = np.ascontiguousarray(np.triu(np.ones((P, P), np.float32)))
    in_maps = []
    for c in range(8):
        b, g = c // 4, c % 4
        sl = slice(E * g, E * (g + 1))
        xs = _strip(x[b].T.astype(np.float32), DC)  # [P, DC, S]
        xs = np.ascontiguousarray(
            xs.reshape(P, DC, NQ, QC).transpose(0, 2, 1, 3)
        )  # [P, NQ, DC, QC], chunk-major
        in_maps.append(
            {
                "xT": xs,
                "wqT": _strip((Wq[sl, :] * 0.125).T.astype(np.float32), DC),
                "wkT": _strip(Wk[sl, :].T.astype(np.float32), DC),
                "wvT": _strip(Wv[sl, :].T.astype(np.float32), DC),
                "woT": _strip(Wo[:, sl].T.astype(np.float32), 2),
                "tri": tri,
            }
        )
    return in_maps


def kernel(x, Wq, Wk, Wv, Wo, bo, _run_kwargs=None):
    x, Wq, Wk, Wv, Wo, bo = (
        np.asarray(a, dtype=np.float32) for a in (x, Wq, Wk, Wv, Wo, bo)
    )
    nc = build_nc()
    in_maps = make_in_maps(x, Wq, Wk, Wv, Wo)
    res = run_bass_kernel_spmd(
        nc, in_maps, core_ids=list(range(8)), **(_run_kwargs or {})
    )
    out = np.zeros((2, S, DD), dtype=np.float32)
    for c in range(8):
        out[c // 4] += res.results[c]["out"]
    out += bo[None, None, :]
    if _run_kwargs:
        kernel.last_results = res
    return out
